# revision 32
# baseline (speedup 1.0000x reference)
"""Child-sum TreeLSTM encoder over the deterministic 8-ary tree (N=200000,
D=256, 7 levels), distributed over 8 Trainium2 NeuronCores.

Sharding: the 512 "units" (unit j = 64 consecutive L5 nodes + their 512 L6
children + 8 L4 parents + 1 L3 grandparent) are dealt to cores so that every
core owns 8 whole superblocks (superblock = 8 units = 1 L2 node's subtree),
interleaved stride-8 so per-core work is uniform.  All parent<-child
aggregation is core-local up to and including the 64 L2 nodes (levels 6..2,
199,927 of the 200,000 nodes); the 9 top nodes (8 L1 + root) are finished on
the host from the device-computed L2 (h, c) states, so the device program
has no collectives and the 8 cores run fully independently.

v3 organization (vs the v1 baseline):
  * everything is float16 on the wire and in SBUF (PSUM accumulation stays
    fp32): halves DMA traffic, and fp16 matmuls run at the same 1 col/cycle
    as fp32r while unlocking the DVE 2x packed mode for element-wise ops.
  * the ENTIRE per-core x (25160 cols = ~100 KiB/partition) is DMAd into
    SBUF up front in a few large chunks, so no load ever contends with a
    compute-dependent store on the queue and the PE never waits for input.
  * the per-edge forget-gate term x_parent @ W_f is folded into the f-gate
    matmul itself using a stride-0 (broadcast-by-8) moving AP, removing the
    xf precompute phase and the per-tile DVE broadcast-add.
  * element-wise multiplies run on DVE (fp16 2x mode), not GPSIMD ("Pool",
    0.42 efficiency); the 8-child segment sums are packed fp16 tree-adds on
    DVE instead of TensorReduce (which has no fast mode).
"""

import numpy as np

# ---------------------------------------------------------------- constants
N = 200_000
D = 256
BR = 8
S = [0, 1, 9, 73, 585, 4681, 37449]   # level start offsets (levels 0..6)
UNITS = 512                            # total units; unit j <-> L3 node 73+j
N_CORES = 8
FULL_SLOTS = 40                        # phase-A child slots per core
T = 512                                # nodes per phase-A slot / phase-B tile
L5_PC, L4_PC, L3_PC, L2_PC = 4096, 512, 64, 8   # per-core node counts
A_COLS = FULL_SLOTS * T                # 20480

# packed input xw column offsets: x sections then weights (all fp16)
OFF_A = 0
OFF_B = OFF_A + A_COLS                 # 20480
OFF_C = OFF_B + L5_PC                  # 24576
OFF_D = OFF_C + L4_PC                  # 25088
OFF_E = OFF_D + L3_PC                  # 25152
X_COLS = OFF_E + L2_PC                 # 25160
OFF_WIOU = X_COLS                      # 25160
OFF_UIOU = OFF_WIOU + 3 * D            # 25928
OFF_WF = OFF_UIOU + 3 * D              # 26696
OFF_UF = OFF_WF + D                    # 26952
XW_COLS = OFF_UF + D                   # 27208

# packed output hAll: h sections A|B|C|D|E (fp16) then c of the 8 L2
# nodes stored as fp32 bitcast into 2*L2_PC fp16 columns (the host top-level
# computation needs c_L2 at full precision: |c_L2| is large, so fp16
# quantization there dominated the root error)
OFF_CE = X_COLS                        # 25160
H_COLS = OFF_CE + 2 * L2_PC            # 25176

_STATE = {}


# ------------------------------------------------------- host-side indexing
def _units_of(core):
    return [8 * (core + 8 * k) + jj for k in range(8) for jj in range(8)]


def _index_maps():
    """Per-core global node-id arrays for each packed segment (-1 = pad)."""
    if "maps" in _STATE:
        return _STATE["maps"]
    maps = []
    for c in range(N_CORES):
        units = _units_of(c)
        idsA = np.full(A_COLS, -1, dtype=np.int64)
        for s in range(FULL_SLOTS):
            j = units[s]
            base = S[6] + T * j
            hi = min(N, base + T)
            if base < N:
                n = hi - base
                idsA[s * T:s * T + n] = np.arange(base, hi)
        idsB = np.concatenate([np.arange(S[5] + 64 * j, S[5] + 64 * (j + 1)) for j in units])
        idsC = np.concatenate([np.arange(S[4] + 8 * j, S[4] + 8 * (j + 1)) for j in units])
        idsD = np.array([S[3] + j for j in units], dtype=np.int64)
        idsE = np.array([S[2] + c + 8 * k for k in range(8)], dtype=np.int64)
        maps.append(dict(A=idsA, B=idsB, C=idsC, D=idsD, E=idsE))
    _STATE["maps"] = maps
    return maps


# ---------------------------------------------------------------- program
def _build_program(null=False, repeats=1):
    import concourse.bacc as bacc
    import concourse.tile as tile
    import concourse.mybir as mybir

    F16 = mybir.dt.float16
    F32 = mybir.dt.float32
    AF = mybir.ActivationFunctionType
    OP = mybir.AluOpType

    nc = bacc.Bacc(num_devices=N_CORES)

    # ---- external I/O (per-core shapes; host packs per core, fp16)
    xw = nc.dram_tensor("xw", [D, XW_COLS], F16, kind="ExternalInput").ap()
    hAll = nc.dram_tensor("hAll", [D, H_COLS], F16, kind="ExternalOutput").ap()

    if null:
        # same I/O signature, trivial compute: isolates dispatch overhead
        with tile.TileContext(nc) as tc:
            with tc.tile_pool(name="p", bufs=1) as p:
                t0 = p.tile([128, 2, T], F16, tag="xk")
                nc.sync.dma_start(out=t0[:, :, :T],
                                  in_=xw[:, 0:T].rearrange("(k p) n -> p k n", p=128))
                h0t = p.tile([128, 2, T], F16, tag="h")
                nc.vector.tensor_copy(out=h0t[:], in_=t0[:])
                nc.gpsimd.dma_start(out=hAll[:, 0:T].rearrange("(k p) n -> p k n", p=128),
                                    in_=h0t[:, :, :T])
        nc.compile()
        return nc

    with tile.TileContext(nc) as tc:
        import contextlib
        with contextlib.ExitStack() as ctx:
            wp = ctx.enter_context(tc.tile_pool(name="wts", bufs=1))
            acc = ctx.enter_context(tc.tile_pool(name="acc", bufs=1))
            gp = ctx.enter_context(tc.tile_pool(name="gates", bufs=4))
            tcp = ctx.enter_context(tc.tile_pool(name="tcs", bufs=2))
            hp = ctx.enter_context(tc.tile_pool(name="hc", bufs=4))
            fp = ctx.enter_context(tc.tile_pool(name="fs", bufs=2))
            bp = ctx.enter_context(tc.tile_pool(name="bht", bufs=1))
            trp = ctx.enter_context(tc.tile_pool(name="tree", bufs=2))
            pio = ctx.enter_context(tc.tile_pool(name="iou", bufs=2, space="PSUM"))
            pxu = ctx.enter_context(tc.tile_pool(name="ugate", bufs=1, space="PSUM"))
            pfp = ctx.enter_context(tc.tile_pool(name="fpre", bufs=1, space="PSUM"))

            # Tiles (weights, x residency, accumulators) are allocated once
            # and reused by every repetition.  The timing variant's reps
            # re-emit ALL work (weight/x DMAs included, with identical
            # values), and reps pipeline into each other with no barrier:
            # each rep's C/D/E tiles are interleaved into the next rep's
            # leading leaf slots so the engines stay busy through the
            # level-drain, and the reload DMAs are spread across the rep.
            wiou_sb = wp.tile([128, 2, 6, 128], F16, tag="wiou")
            uiou_sb = wp.tile([128, 2, 6, 128], F16, tag="uiou")
            wf_sb = wp.tile([128, 2, 2, 128], F16, tag="wf")
            uf_sb = wp.tile([128, 2, 2, 128], F16, tag="uf")
            xall = acc.tile([128, 2, X_COLS], F16, tag="xall")

            def load_weights():
                for k in range(2):
                    rows = slice(128 * k, 128 * (k + 1))
                    nc.sync.dma_start(out=wiou_sb[:, k, :, :],
                                      in_=xw[rows, OFF_WIOU:OFF_WIOU + 3 * D].rearrange("p (m q) -> p m q", q=128))
                    nc.sync.dma_start(out=uiou_sb[:, k, :, :],
                                      in_=xw[rows, OFF_UIOU:OFF_UIOU + 3 * D].rearrange("p (m q) -> p m q", q=128))
                    nc.sync.dma_start(out=wf_sb[:, k, :, :],
                                      in_=xw[rows, OFF_WF:OFF_WF + D].rearrange("p (m q) -> p m q", q=128))
                    nc.sync.dma_start(out=uf_sb[:, k, :, :],
                                      in_=xw[rows, OFF_UF:OFF_UF + D].rearrange("p (m q) -> p m q", q=128))

            def load_chunk(lo, hi):
                nc.sync.dma_start(
                    out=xall[:, :, lo:hi],
                    in_=xw[:, lo:hi].rearrange("(k p) n -> p k n", p=128))

            # ---------------- persistent accumulators
            fc4 = acc.tile([128, 2, L4_PC], F32, tag="fc4")
            ht4 = acc.tile([128, 2, L4_PC], F16, tag="ht4")
            fc3 = acc.tile([128, 2, L3_PC], F32, tag="fc3")
            ht3 = acc.tile([128, 2, L3_PC], F16, tag="ht3")
            fc2 = acc.tile([128, 2, L2_PC], F32, tag="fc2")
            ht2 = acc.tile([128, 2, L2_PC], F16, tag="ht2")
            fc5_0 = bp.tile([128, 2, T], F32, tag="fc5_0")
            fc5_1 = bp.tile([128, 2, T], F32, tag="fc5_1")
            ht5_0 = bp.tile([128, 2, T], F16, tag="ht5_0")
            ht5_1 = bp.tile([128, 2, T], F16, tag="ht5_1")
            fc5 = [fc5_0, fc5_1]
            ht5 = [ht5_0, ht5_1]

            # --------- staged node processing (software-pipelined stream)
            def head_s1(x_lo, n, ht_rhs):
                """PE gate matmuls + Act sigmoids/tanh -> (gio, tuu).

                The u gate of BOTH halves accumulates into one shared PSUM
                tile so a single tanh covers it (one 1024-elem Act
                instruction instead of two 512-elem ones); u matmuls go
                first so the u-tile ring wait lands at the tile boundary
                where the io-ring wait already is."""
                gio = gp.tile([128, 2, 2, T], F16, tag="gio")
                tuu = gp.tile([128, 2, T], F16, tag="tuu")
                last = ht_rhs is None
                psu = pxu.tile([128, 2, T], F32, tag="psu")
                for half in range(2):
                    m = 4 + half
                    for k in range(2):
                        nc.tensor.matmul(psu[:, half, :n], wiou_sb[:, k, m, :],
                                         xall[:, k, x_lo:x_lo + n],
                                         start=(k == 0), stop=(last and k == 1))
                    if ht_rhs is not None:
                        for k in range(2):
                            nc.tensor.matmul(psu[:, half, :n], uiou_sb[:, k, m, :], ht_rhs[:, k, :],
                                             start=False, stop=(k == 1))
                nc.scalar.activation(tuu[:, :, :n], psu[:, :, :n], AF.Tanh)
                for half in range(2):
                    ps = pio.tile([128, 2, T], F32, tag="ps2")
                    for g, m in enumerate((half, 2 + half)):  # i, o
                        for k in range(2):
                            nc.tensor.matmul(ps[:, g, :n], wiou_sb[:, k, m, :],
                                             xall[:, k, x_lo:x_lo + n],
                                             start=(k == 0), stop=(last and k == 1))
                        if ht_rhs is not None:
                            for k in range(2):
                                nc.tensor.matmul(ps[:, g, :n], uiou_sb[:, k, m, :], ht_rhs[:, k, :],
                                                 start=False, stop=(k == 1))
                    nc.scalar.activation(gio[:, :, half, :n], ps[:, :, :n], AF.Sigmoid)
                return gio, tuu

            def head_s2(gio, tuu, fc_in, n, hi):
                """DVE: c = sig(i)*tanh(u) [+ fc]."""
                c = hp.tile([128, 2, T], F32 if hi else F16, tag="c")
                nc.vector.tensor_tensor(out=c[:, :, :n], in0=gio[:, 0, :, :n],
                                        in1=tuu[:, :, :n], op=OP.mult)
                if fc_in is not None:
                    nc.vector.tensor_tensor(out=c[:, :, :n], in0=c[:, :, :n],
                                            in1=fc_in[:, :, :], op=OP.add)
                return c

            def head_s3(c, n):
                """Act: tanh(c)."""
                tc_ = tcp.tile([128, 2, T], F16, tag="tc")
                nc.scalar.activation(tc_[:, :, :n], c[:, :, :n], AF.Tanh)
                return tc_

            def head_s4(gio, tc_, n, out_lo):
                """DVE: h = sig(o)*tanh(c); DMA h out."""
                h = hp.tile([128, 2, T], F16, tag="h")
                nc.vector.tensor_tensor(out=h[:, :, :n], in0=gio[:, 1, :, :n],
                                        in1=tc_[:, :, :n], op=OP.mult)
                nc.sync.dma_start(out=hAll[:, out_lo:out_lo + n].rearrange("(k p) n -> p k n", p=128),
                                  in_=h[:, :, :n])
                return h

            def tail_s1(h, n, ngrp, par_lo):
                """PE: f-pre = h @ Uf + x_parent @ W_f (broadcast by 8)."""
                fpre = pfp.tile([128, 2, T], F32, tag="fpre")
                for m in range(2):
                    for k in range(2):
                        nc.tensor.matmul(fpre[:, m, :n], uf_sb[:, k, m, :], h[:, k, :n],
                                         start=(k == 0), stop=False)
                    for k in range(2):
                        xpar = (xall[:, k, par_lo:par_lo + ngrp]
                                .rearrange("p (g o) -> p g o", o=1)
                                .broadcast_to([128, ngrp, 8]))
                        nc.tensor.matmul(fpre[:, m, :n], wf_sb[:, k, m, :], xpar,
                                         start=False, stop=(k == 1))
                return fpre

            def tail_s3(fpre, n, hi):
                """Act: f = sigmoid(f-pre)."""
                f = fp.tile([128, 2, T], F32 if hi else F16, tag="f")
                nc.scalar.activation(f[:, :, :n], fpre[:, :, :n], AF.Sigmoid)
                return f

            def tail_s4(f, c, n):
                """DVE: f *= c (in place; fp16 2x on leaf tiles)."""
                nc.vector.tensor_tensor(out=f[:, :, :n], in0=f[:, :, :n],
                                        in1=c[:, :, :n], op=OP.mult)
                return f

            def tree_sum8(src, n, ngrp, dst, dt):
                """DVE packed tree-add: dst[g] = sum_e src[g*8+e]."""
                g = src[:, :, :n].rearrange("p m (g e) -> p m g e", e=8)
                t1 = trp.tile([128, 2, T // 8, 4], dt, tag="tr1")
                nc.vector.tensor_tensor(out=t1[:, :, :ngrp, :], in0=g[:, :, :, 0:4],
                                        in1=g[:, :, :, 4:8], op=OP.add)
                t2 = trp.tile([128, 2, T // 8, 2], dt, tag="tr2")
                nc.vector.tensor_tensor(out=t2[:, :, :ngrp, :], in0=t1[:, :, :ngrp, 0:2],
                                        in1=t1[:, :, :ngrp, 2:4], op=OP.add)
                nc.vector.tensor_tensor(out=dst, in0=t2[:, :, :ngrp, 0],
                                        in1=t2[:, :, :ngrp, 1], op=OP.add)

            def tail_s5(fj, h, n, ngrp, fc_dst, ht_dst, dst_lo):
                """DVE: grouped segment sums fc and h-tilde."""
                tree_sum8(fj, n, ngrp, fc_dst[:, :, dst_lo:dst_lo + ngrp], F32)
                tree_sum8(h, n, ngrp, ht_dst[:, :, dst_lo:dst_lo + ngrp], F16)

            # ------------- unified tile stream across reps, skewed pipeline
            def a_slot(s):
                t, r = s // 8, (s // 8) % 2
                return (OFF_A + s * T, T, None, None, OFF_A + s * T,
                        (64, fc5[r], ht5[r], (s - t * 8) * 64, OFF_B + s * 64))

            def b_tile(t):
                r = t % 2
                if t < 5:
                    return (OFF_B + t * T, T, ht5[r], fc5[r], OFF_B + t * T,
                            (64, fc4, ht4, t * 64, OFF_C + t * 64))
                return (OFF_B + t * T, T, None, None, OFF_B + t * T,
                        (64, fc4, ht4, t * 64, OFF_C + t * 64))

            def cde_entries():
                return [
                    (OFF_C, T, ht4, fc4, OFF_C, (64, fc3, ht3, 0, OFF_D)),
                    (OFF_D, L3_PC, ht3, fc3, OFF_D, (8, fc2, ht2, 0, OFF_E)),
                    (OFF_E, L2_PC, ht2, fc2, OFF_E, None),
                ]

            stream = []
            hooks = {}

            def add_hook(idx, fn):
                hooks.setdefault(idx, []).append(fn)

            for rep in range(repeats):
                g = len(stream)
                inject = cde_entries() if rep > 0 else []
                # reload DMAs spread over the rep start (values identical
                # across reps, so ordering vs. stale readers is value-safe)
                add_hook(g, load_weights)
                add_hook(g, lambda: load_chunk(OFF_A, OFF_A + 2 * T))
                add_hook(g, lambda: load_chunk(OFF_A + 2 * T, OFF_A + 4 * T))
                add_hook(g, lambda: load_chunk(OFF_B, OFF_B + L5_PC))
                add_hook(g + 4, lambda: load_chunk(OFF_C, X_COLS))
                add_hook(g + 4, lambda: load_chunk(OFF_A + 4 * T, OFF_A + 12 * T))
                add_hook(g + 12, lambda: load_chunk(OFF_A + 12 * T, OFF_A + 24 * T))
                add_hook(g + 20, lambda: load_chunk(OFF_A + 24 * T, OFF_A + 40 * T))
                for t in range(5):
                    for s in range(t * 8, t * 8 + 8):
                        stream.append(a_slot(s))
                        if t == 0 and inject and s in (2, 5, 8):
                            stream.append(inject.pop(0))
                        if s == 8 and inject:
                            stream.append(inject.pop(0))
                        if t >= 1 and s == t * 8 + 3:
                            stream.append(b_tile(t - 1))
                stream += [b_tile(5), b_tile(6), b_tile(7), b_tile(4)]

            SKEW = 3
            n_str = len(stream)
            st = {}   # per-index pipeline state

            def emit_s1(i):
                x_lo, n, ht_rhs, _, _, _ = stream[i]
                rhs = ht_rhs[:, :, :] if ht_rhs is not None else None
                st[i] = {"g1": head_s1(x_lo, n, rhs)}

            def emit_headfin(i):
                x_lo, n, _, fc_in, out_lo, _ = stream[i]
                gio, tuu = st[i]["g1"]
                fci = fc_in[:, :, :] if fc_in is not None else None
                c = head_s2(gio, tuu, fci, n, hi=(x_lo >= OFF_B))
                tc_ = head_s3(c, n)
                h = head_s4(gio, tc_, n, out_lo)
                st[i]["c"], st[i]["h"] = c, h
                if out_lo == OFF_E and n == L2_PC:
                    # phase E: also emit c of the 8 L2 nodes, fp32 bitcast
                    nc.sync.dma_start(
                        out=hAll[:, OFF_CE:OFF_CE + 2 * L2_PC].rearrange("(k p) n -> p k n", p=128),
                        in_=c[:, :, :L2_PC].bitcast(F16))

            def emit_tail(i):
                _, n, _, _, _, tail = stream[i]
                if tail is None:
                    return
                ngrp, fc_dst, ht_dst, dst_lo, par_lo = tail
                h, c = st[i]["h"], st[i]["c"]
                fpre = tail_s1(h, n, ngrp, par_lo)
                f = tail_s3(fpre, n, hi=(stream[i][0] >= OFF_B))
                fj = tail_s4(f, c, n)
                tail_s5(fj, h, n, ngrp, fc_dst, ht_dst, dst_lo)

            for i in range(min(SKEW, n_str)):
                for fn in hooks.pop(i, []):
                    fn()
                emit_s1(i)
            emit_headfin(0)
            for i in range(n_str):
                if i + SKEW < n_str:
                    for fn in hooks.pop(i + SKEW, []):
                        fn()
                    emit_s1(i + SKEW)
                if i + 1 < n_str:
                    emit_headfin(i + 1)
                emit_tail(i)
                st.pop(i - 2, None)

            # final rep's C/D/E: emitted serially (each depends on tails
            # emitted at the stream end; nothing is left to overlap with)
            for ent in cde_entries():
                stream.append(ent)
                j = len(stream) - 1
                emit_s1(j)
                emit_headfin(j)
                emit_tail(j)

    nc.compile()
    return nc


R_TIMING = 16  # kernel repetitions per NEFF execution in the timing variant


def _get_nc(variant="main"):
    key = "nc_" + variant
    if key not in _STATE:
        if variant == "null":
            _STATE[key] = _build_program(null=True)
        elif variant == "timing":
            _STATE[key] = _build_program(repeats=R_TIMING)
        else:
            _STATE[key] = _build_program()
    return _STATE[key]


# ---------------------------------------------------------------- host glue
def _pack_inputs(x, W_iou, U_iou, W_f, U_f):
    maps = _index_maps()
    x = np.asarray(x, dtype=np.float32)
    x_pad = np.vstack([x, np.zeros((1, D), np.float32)])  # row N = zeros (for -1 ids)
    w_block = np.concatenate([
        np.asarray(W_iou, np.float32), np.asarray(U_iou, np.float32),
        np.asarray(W_f, np.float32), np.asarray(U_f, np.float32)], axis=1).astype(np.float16)
    in_maps = []
    for c in range(N_CORES):
        m = maps[c]
        xsec = np.concatenate([x_pad[m[name]].T for name in ("A", "B", "C", "D", "E")],
                              axis=1).astype(np.float16)
        in_maps.append({"xw": np.ascontiguousarray(
            np.concatenate([xsec, w_block], axis=1))})
    return in_maps


def _sig(z):
    return 1.0 / (1.0 + np.exp(-z))


def _host_top(x, W_iou, U_iou, b_iou, W_f, U_f, b_f, hL2, cL2):
    """Finish L1 nodes 1..8 and the root on the host from L2 (h, c).

    hL2/cL2 are [64, D] for global nodes 9..72; children of L1 node p are
    nodes 8p+1..8p+8, i.e. L2 block p-1."""
    hc = hL2.reshape(8, 8, D)
    cc = cL2.reshape(8, 8, D)
    hL1 = np.zeros((8, D), np.float32)
    cL1 = np.zeros((8, D), np.float32)
    out = np.zeros((9, D), np.float32)

    def node(xrow, ch_h, ch_c):
        h_t = ch_h.sum(0)
        f = _sig(xrow @ W_f + b_f + ch_h @ U_f)
        fc = (f * ch_c).sum(0)
        iou = xrow @ W_iou + b_iou + h_t @ U_iou
        cg = _sig(iou[:D]) * np.tanh(iou[2 * D:]) + fc
        hg = _sig(iou[D:2 * D]) * np.tanh(cg)
        return hg, cg

    for p in range(1, 9):
        hL1[p - 1], cL1[p - 1] = node(x[p], hc[p - 1], cc[p - 1])
        out[p] = hL1[p - 1]
    out[0], _ = node(x[0], hL1, cL1)
    return out


def _unpack_outputs(results, x, W_iou, U_iou, b_iou, W_f, U_f, b_f):
    maps = _index_maps()
    out = np.empty((N, D), dtype=np.float32)
    sections = (("A", OFF_A, A_COLS), ("B", OFF_B, L5_PC), ("C", OFF_C, L4_PC),
                ("D", OFF_D, L3_PC), ("E", OFF_E, L2_PC))
    hL2 = np.empty((64, D), np.float32)
    cL2 = np.empty((64, D), np.float32)
    for c in range(N_CORES):
        m = maps[c]
        hT = results[c]["hAll"].T.astype(np.float32)   # [H_COLS, D]
        for name, lo, ncols in sections:
            ids = m[name]
            valid = ids >= 0
            out[ids[valid]] = hT[lo:lo + ncols][valid]
        csec = np.ascontiguousarray(
            results[c]["hAll"][:, OFF_CE:OFF_CE + 2 * L2_PC]).view(np.float32)
        for k in range(8):                 # L2 node S[2]+c+8k -> index c+8k
            hL2[c + 8 * k] = hT[OFF_E + k]
            cL2[c + 8 * k] = csec[:, k]
    out[0:9] = _host_top(np.asarray(x, np.float32)[0:9],
                         np.asarray(W_iou, np.float32), np.asarray(U_iou, np.float32),
                         np.asarray(b_iou, np.float32),
                         np.asarray(W_f, np.float32), np.asarray(U_f, np.float32),
                         np.asarray(b_f, np.float32), hL2, cL2)
    return out


# ---------------------------------------------------------------- jitted runner
def _get_runner(variant="main"):
    """Compile the SPMD executable once (fast-dispatch); returns helpers."""
    rkey = "runner_" + variant
    if rkey in _STATE:
        return _STATE[rkey]
    import jax
    import jax.numpy as jnp
    from jax.sharding import Mesh, PartitionSpec, NamedSharding
    from jax.experimental.shard_map import shard_map
    import concourse.mybir as mybir
    from concourse import bass2jax

    nc = _get_nc(variant)
    bass2jax.install_neuronx_cc_hook()

    partition_name = nc.partition_id_tensor.name if nc.partition_id_tensor else None
    in_names, out_names, out_avals = [], [], []
    for alloc in nc.m.functions[0].allocations:
        if not isinstance(alloc, mybir.MemoryLocationSet):
            continue
        name = alloc.memorylocations[0].name
        if alloc.kind == "ExternalInput":
            if name != partition_name:
                in_names.append(name)
        elif alloc.kind == "ExternalOutput":
            out_names.append(name)
            out_avals.append(jax.core.ShapedArray(tuple(alloc.tensor_shape),
                                                  mybir.dt.np(alloc.dtype)))
    n_params = len(in_names)
    n_outs = len(out_names)
    all_names = in_names + out_names
    if partition_name is not None:
        all_names = all_names + [partition_name]

    def _body(*args):
        operands = list(args)
        if partition_name is not None:
            operands.append(bass2jax.partition_id_tensor())
        outs = bass2jax._bass_exec_p.bind(
            *operands,
            out_avals=tuple(out_avals),
            in_names=tuple(all_names),
            out_names=tuple(out_names),
            lowering_input_output_aliases=(),
            sim_require_finite=True,
            sim_require_nnan=True,
            nc=nc,
        )
        return tuple(outs)

    devices = jax.devices()[:N_CORES]
    mesh = Mesh(np.asarray(devices), ("core",))
    sharding = NamedSharding(mesh, PartitionSpec("core"))
    donate = tuple(range(n_params, n_params + n_outs))
    state = {}

    def _fast(args):
        if "c" not in state:
            def compile_fn():
                jitted = jax.jit(
                    shard_map(_body, mesh=mesh,
                              in_specs=(PartitionSpec("core"),) * (n_params + n_outs),
                              out_specs=(PartitionSpec("core"),) * n_outs,
                              check_rep=False),
                    donate_argnums=donate, keep_unused=True)
                return jitted.lower(*args).compile()
            state["c"] = bass2jax.fast_dispatch_compile(compile_fn)
        return state["c"]

    def make_zero_outs():
        outs = []
        for a in out_avals:
            shape = (N_CORES * a.shape[0], *a.shape[1:])
            outs.append(jax.jit(lambda s=shape, d=a.dtype: jnp.zeros(s, d),
                                out_shardings=sharding)())
        return outs

    def put_inputs(in_maps):
        return [jax.device_put(
            np.concatenate([np.asarray(in_maps[c][n]) for c in range(N_CORES)], axis=0),
            sharding) for n in in_names]

    def run_nosync(dev_inputs, zero_outs):
        return _fast((*dev_inputs, *zero_outs))(*dev_inputs, *zero_outs)

    def run(dev_inputs, zero_outs):
        out_arrs = run_nosync(dev_inputs, zero_outs)
        jax.block_until_ready(out_arrs)
        return out_arrs

    def to_results(out_arrs):
        return [
            {name: np.asarray(out_arrs[i]).reshape(N_CORES, *out_avals[i].shape)[c]
             for i, name in enumerate(out_names)}
            for c in range(N_CORES)
        ]

    r = dict(run=run, run_nosync=run_nosync, put_inputs=put_inputs,
             make_zero_outs=make_zero_outs,
             to_results=to_results, out_names=out_names, in_names=in_names)
    _STATE[rkey] = r
    return r


def _run(in_maps):
    r = _get_runner()
    dev_inputs = r["put_inputs"](in_maps)
    out_arrs = r["run"](dev_inputs, r["make_zero_outs"]())
    return r["to_results"](out_arrs)


def kernel(x, W_iou, U_iou, b_iou, W_f, U_f, b_f, parent_idx, level, num_levels):
    b_iou = np.asarray(b_iou)
    b_f = np.asarray(b_f)
    parent_idx = np.asarray(parent_idx)
    expect_parent = (np.arange(1, N, dtype=np.int64) - 1) // BR
    assert parent_idx.shape == (N - 1,) and np.array_equal(parent_idx, expect_parent), \
        "kernel is specialized to the deterministic 8-ary tree parent(i)=(i-1)//8"
    assert not b_iou.any() and not b_f.any(), "device program assumes zero biases"
    in_maps = _pack_inputs(x, W_iou, U_iou, W_f, U_f)
    results = _run(in_maps)
    return _unpack_outputs(results, x, W_iou, U_iou, b_iou, W_f, U_f, b_f)
